# revision 1
# baseline (speedup 1.0000x reference)
"""DualHOILoss Trainium2 kernel (8 NeuronCores, pure data parallel over batch).

Math (per batch b, point p, object o in verts+anchors):
    t_p = (basis_p + delta_p) / s + m           (u = basis + delta, w_o = o - m)
    d2[p,o] = |t_p - o|^2 = u.(-2w/s) + |w|^2 + |u|^2/s^2
computed as a fp32r matmul (lhsT rows [ux,uy,uz,1], rhs rows
[-2wx/s,-2wy/s,-2wz/s,|w|^2]) plus a per-point |u|^2/s^2 correction.
Tiles are processed in PAIRS sharing one K=8 lhsT; the two rhs variants
zero the other tile's 4 rows so each matmul sees only its own points.

Per 128-point tile: PE writes vert-d2 into PSUM bankA (N1 cols) and
[anchor-d2, remaining vert-d2] into PSUM bankB; ACT drains bankB pairs to
SBUF; DVE tensor_tensor_scan(min,min) folds bankA against the SBUF copy
(INF-padded) into a running min; GPSIMD extracts mins and masks anchor
columns with the one-hot of the anchor index.  Batch-level DVE/ACT ops turn
the min-d2 / selected-d2 slabs into the two squared-error partial sums, and
a final fp32 matmul against ones reduces partitions.  Host sums 8x4 partials.

Point tiling uses the SBUF-natural index map p = 32*q + tau (partition q,
tile tau) so every DMA is contiguous.
"""

import numpy as np

B, P, A, V = 16, 4096, 32, 778
NCORES = 8
BPC = B // NCORES      # batches per core
NT = P // 128          # 32 point tiles per batch
N1 = 480               # vert columns in PSUM bankA (scanned directly)
N2 = V - N1            # vert columns in bankB (drained to SBUF)
NB = A + N2            # bankB columns (anchors first)
C2W = 512              # per-tile width of the SBUF drain buffer
INF = 3.0e38

_CACHE = {}


def _build_program():
    import concourse.bacc as bacc
    import concourse.mybir as mybir
    from concourse import tile

    f32 = mybir.dt.float32
    f32r = mybir.dt.float32r
    AF = mybir.ActivationFunctionType
    ALU = mybir.AluOpType
    AX = mybir.AxisListType

    nc = bacc.Bacc(None, target_bir_lowering=False)

    chc_d = nc.dram_tensor("chc", [BPC, 128, 6 * NT + NT], f32, kind="ExternalInput")
    basis_d = nc.dram_tensor("basis", [128, 3 * NT], f32, kind="ExternalInput")
    obj_d = nc.dram_tensor("obj", [BPC, 128, 24], f32, kind="ExternalInput")
    sbc_d = nc.dram_tensor("sbc", [128, 12], f32, kind="ExternalInput")
    out_d = nc.dram_tensor("partials", [4, 1], f32, kind="ExternalOutput")

    cpack = np.zeros((128, 161), np.float32)
    cpack[:, 0:128] = np.eye(128, dtype=np.float32)
    cpack[:, 128:160] = np.tile(np.arange(32, dtype=np.float32), (128, 1))
    cpack[:, 160] = 1.0
    cpack_d = nc.inline_tensor(cpack, "cpack")
    zpad_d = nc.inline_tensor(np.zeros((8, V + A), np.float32), "zpad")
    infpad_d = nc.inline_tensor(
        np.full((128, 2 * (C2W - NB)), INF, np.float32), "infpad")

    NVC = 6            # full 128-vert chunks (768 verts)
    VT = V - 128 * NVC  # tail verts (10)

    with tile.TileContext(nc) as tc:
        with (
            tc.tile_pool(name="sb", bufs=1) as sb,          # persistent
            tc.tile_pool(name="sb2", bufs=2) as sb2,        # per-batch
            tc.tile_pool(name="sbl", bufs=4) as sbl,        # per-tile small
            tc.tile_pool(name="psA", bufs=2, space="PSUM") as psA,
            tc.tile_pool(name="psB", bufs=2, space="PSUM") as psB,
            tc.tile_pool(name="pst", bufs=2, space="PSUM") as pst,
        ):
            sbc = sb.tile([128, 12], f32, tag="sbc")
            nc.scalar.dma_start(sbc[:], sbc_d[:])
            objbufs = []
            for i in range(BPC):
                ob = sb.tile([128, 24], f32, tag=f"obj_{i}")
                nc.scalar.dma_start(ob[:], obj_d[i])
                objbufs.append(ob)
            cpk = sb.tile([128, 161], f32, tag="cpk")
            nc.scalar.dma_start(cpk[:], cpack_d[:])
            ident = cpk[:, 0:128]
            iota = cpk[:, 128:160]
            ones = cpk[:, 160:161]

            # host-prepared launch scalars, pre-broadcast to 128 partitions
            neg2s_bc = sbc[:, 6:7]
            inv_s2_bc = sbc[:, 7:8]
            b_negeps = sbc[:, 8:9]
            b_poseps = sbc[:, 9:10]
            b_negexp = sbc[:, 10:11]

            basis = sb.tile([128, 3 * NT], f32, tag="basis")
            nc.sync.dma_start(basis[:], basis_d[:])

            part = sb.tile([128, 4], f32, tag="part")

            # persistent ping/pong drain buffers; INF pads written once
            c2bufs = []
            for i in range(2):
                c2b = sb.tile([128, 2 * C2W], f32, tag=f"c2_{i}")
                nc.vector.memset(c2b[:, NB:C2W], INF)
                nc.vector.memset(c2b[:, C2W + NB :], INF)
                c2bufs.append(c2b)

            # persistent per-batch rhs variants; zero halves written once
            rhsA_bufs, rhsB_bufs = [], []
            for i in range(BPC):
                ra = sb.tile([8, V + A], f32r, tag=f"rhsA_{i}")
                nc.sync.dma_start(ra[:].bitcast(f32), zpad_d[:])
                rhsA_bufs.append(ra)
                rb = sb.tile([8, V + A], f32r, tag=f"rhsB_{i}")
                nc.sync.dma_start(rb[:].bitcast(f32), zpad_d[:])
                rhsB_bufs.append(rb)

            for b in range(BPC):
                # ---------------- per-batch loads ----------------
                chc = sb2.tile([128, 6 * NT + NT], f32, tag="chc")
                nc.sync.dma_start(chc[:], chc_d[b])
                ch = chc[:, 0 : 6 * NT]
                hc = chc[:, 6 * NT :]

                # ---------------- rhs build ----------------
                obj = objbufs[b]
                vnat = obj[:, 0:18]
                vtail = obj[0:VT, 18:21]
                anat = obj[0:A, 21:24]
                mrep = sbc[:, 3 * b : 3 * b + 3]

                wnat = sb2.tile([128, 3 * NVC], f32, tag="wnat")
                nc.gpsimd.tensor_tensor(
                    wnat[:].rearrange("p (c d) -> p c d", d=3),
                    vnat.rearrange("p (c d) -> p c d", d=3),
                    mrep.unsqueeze(1).broadcast_to([128, NVC, 3]),
                    op=ALU.subtract,
                )
                wtail = sb2.tile([VT, 3], f32, tag="wtail")
                nc.gpsimd.tensor_tensor(
                    wtail[:], vtail, mrep[0:VT], op=ALU.subtract)
                wanat = sb2.tile([A, 3], f32, tag="wanat")
                nc.gpsimd.tensor_tensor(
                    wanat[:], anat, mrep[0:A], op=ALU.subtract)

                # object scratch: per chunk cols [ -2w/s (3), |w|^2 ]
                s6 = sb2.tile([128, 4 * NVC], f32, tag="s6")
                st = sb2.tile([VT, 4], f32, tag="st")
                sa = sb2.tile([A, 4], f32, tag="sa")
                nc.vector.tensor_scalar_mul(
                    s6[:].rearrange("p (c d) -> p c d", d=4)[:, :, 0:3],
                    wnat[:].rearrange("p (c d) -> p c d", d=3),
                    neg2s_bc,
                )
                nc.vector.tensor_scalar_mul(
                    st[:, 0:3], wtail[:], neg2s_bc[0:VT])
                nc.vector.tensor_scalar_mul(
                    sa[:, 0:3], wanat[:], neg2s_bc[0:A])
                wsq = sb2.tile([128, 3 * NVC], f32, tag="wsq")
                nc.gpsimd.tensor_tensor(wsq[:], wnat[:], wnat[:], op=ALU.mult)
                nc.vector.tensor_reduce(
                    s6[:].rearrange("p (c d) -> p c d", d=4)[:, :, 3:4].squeeze(2),
                    wsq[:].rearrange("p (c d) -> p c d", d=3),
                    axis=AX.X, op=ALU.add,
                )
                wsqt = sb2.tile([VT, 3], f32, tag="wsqt")
                nc.gpsimd.tensor_tensor(wsqt[:], wtail[:], wtail[:], op=ALU.mult)
                nc.vector.tensor_reduce(st[:, 3:4], wsqt[:], axis=AX.X, op=ALU.add)
                wsqa = sb2.tile([A, 3], f32, tag="wsqa")
                nc.gpsimd.tensor_tensor(wsqa[:], wanat[:], wanat[:], op=ALU.mult)
                nc.vector.tensor_reduce(sa[:, 3:4], wsqa[:], axis=AX.X, op=ALU.add)

                # transpose object scratches into rhsA rows 0:4 (f32r)
                rhsA = rhsA_bufs[b]
                rhsB = rhsB_bufs[b]
                for c in range(NVC):
                    Tc = pst.tile([8, 128], f32, tag="tp")
                    nc.tensor.transpose(
                        Tc[0:4, :], s6[:, 4 * c : 4 * c + 4], ident)
                    nc.scalar.activation(
                        rhsA[0:4, 128 * c : 128 * (c + 1)], Tc[0:4, :], AF.Copy)
                Tt = pst.tile([8, 128], f32, tag="tp")
                nc.tensor.transpose(Tt[0:4, 0:VT], st[:], ident[0:VT, 0:VT])
                nc.scalar.activation(
                    rhsA[0:4, 128 * NVC : V], Tt[0:4, 0:VT], AF.Copy)
                Ta = pst.tile([8, 128], f32, tag="tp")
                nc.tensor.transpose(Ta[0:4, 0:A], sa[:], ident[0:A, 0:A])
                nc.scalar.activation(rhsA[0:4, V : V + A], Ta[0:4, 0:A], AF.Copy)
                # variant B = variant A shifted to rows 4:8 (one DMA)
                nc.sync.dma_start(rhsB[4:8, :], rhsA[0:4, :])

                # ---------------- one-hot of anchor idx ----------------
                oh = sb2.tile([128, 32 * NT], f32, tag="oh")
                nc.vector.tensor_tensor(
                    oh[:].rearrange("p (t a) -> p t a", a=32),
                    iota.unsqueeze(1).broadcast_to([128, NT, 32]),
                    ch[:].rearrange("p (t s) -> p t s", s=6)[:, :, 5:6]
                        .broadcast_to([128, NT, 32]),
                    op=ALU.is_equal,
                )

                # ---------------- per-tile main loop ----------------
                u_all = sb2.tile([128, 4 * NT], f32, tag="u_all")
                nc.gpsimd.memset(
                    u_all[:].rearrange("p (t d) -> p t d", d=4)[:, :, 3:4], 1.0)
                mind = sb2.tile([128, NT], f32, tag="mind")
                msel = sb2.tile([128, 32 * NT], f32, tag="msel")

                for kp in range(NT // 2):
                    ptB = psB.tile([128, 1024], f32, tag="ptB")
                    c2 = c2bufs[kp % 2]
                    # paired u-add for both tiles of the pair
                    nc.gpsimd.tensor_tensor(
                        u_all[:, 8 * kp : 8 * kp + 8]
                            .rearrange("p (j d) -> p j d", d=4)[:, :, 0:3],
                        ch[:, 12 * kp : 12 * kp + 12]
                            .rearrange("p (j s) -> p j s", s=6)[:, :, 1:4],
                        basis[:, 6 * kp : 6 * kp + 6]
                            .rearrange("p (j d) -> p j d", d=3),
                        op=ALU.add,
                    )
                    T8 = pst.tile([8, 128], f32, tag="tp")
                    nc.tensor.transpose(
                        T8[:], u_all[:, 8 * kp : 8 * kp + 8], ident)
                    lt8 = sbl.tile([8, 128], f32r, tag="lt8")
                    nc.scalar.activation(lt8[:], T8[:], AF.Copy)

                    ptAs = []
                    for j in range(2):
                        rhs = rhsA if j == 0 else rhsB
                        ptA = psA.tile([128, 512], f32, tag="ptA")
                        ptAs.append(ptA)
                        nc.tensor.matmul(ptA[:, 0:N1], lt8[:], rhs[:, 0:N1],
                                         start=True, stop=True)
                        boff = 512 * j
                        nc.tensor.matmul(ptB[:, boff : boff + A], lt8[:],
                                         rhs[:, V : V + A],
                                         start=True, stop=True)
                        nc.tensor.matmul(ptB[:, boff + A : boff + NB], lt8[:],
                                         rhs[:, N1:V], start=True, stop=True)
                    nc.scalar.activation(
                        c2[:].rearrange("p (j w) -> p j w", j=2)[:, :, 0:NB],
                        ptB[:].rearrange("p (j w) -> p j w", j=2)[:, :, 0:NB],
                        AF.Copy,
                    )
                    junk = sbl.tile([128, 2 * N1], f32, tag="junk")
                    for j in range(2):
                        nc.vector.tensor_tensor_scan(
                            out=junk[:, N1 * j : N1 * (j + 1)],
                            data0=ptAs[j][:, 0:N1],
                            data1=c2[:, C2W * j + A : C2W * j + A + N1],
                            initial=INF, op0=ALU.min, op1=ALU.min,
                        )
                    # paired extract of both running-min tails
                    nc.gpsimd.tensor_copy(
                        mind[:, 2 * kp : 2 * kp + 2],
                        junk[:].rearrange("p (j w) -> p j w", w=N1)[:, :, N1 - 1],
                    )
                    # paired anchor masking
                    nc.gpsimd.tensor_tensor(
                        msel[:, 64 * kp : 64 * kp + 64]
                            .rearrange("p (j a) -> p j a", a=32),
                        c2[:].rearrange("p (j w) -> p j w", j=2)[:, :, 0:A],
                        oh[:, 64 * kp : 64 * kp + 64]
                            .rearrange("p (j a) -> p j a", a=32),
                        op=ALU.mult,
                    )

                # ---------------- batch tails ----------------
                usq = sb2.tile([128, 4 * NT], f32, tag="usq")
                nc.gpsimd.tensor_tensor(
                    usq[:].rearrange("p (t d) -> p t d", d=4)[:, :, 0:3],
                    u_all[:].rearrange("p (t d) -> p t d", d=4)[:, :, 0:3],
                    u_all[:].rearrange("p (t d) -> p t d", d=4)[:, :, 0:3],
                    op=ALU.mult,
                )
                uu = sb2.tile([128, NT], f32, tag="uu")
                nc.vector.tensor_reduce(
                    uu[:],
                    usq[:].rearrange("p (t d) -> p t d", d=4)[:, :, 0:3],
                    axis=AX.X, op=ALU.add,
                )
                selr = sb2.tile([128, NT], f32, tag="selr")
                nc.vector.tensor_reduce(
                    selr[:],
                    msel[:].rearrange("p (t a) -> p t a", a=32),
                    axis=AX.X, op=ALU.add,
                )
                # mind2 = mind + uu/s^2 ; d2sel = selr + uu/s^2
                mind2 = sb2.tile([128, NT], f32, tag="mind2")
                nc.vector.scalar_tensor_tensor(
                    out=mind2[:], in0=uu[:], scalar=inv_s2_bc, in1=mind[:],
                    op0=ALU.mult, op1=ALU.add,
                )
                d2sel = sb2.tile([128, NT], f32, tag="d2sel")
                nc.vector.scalar_tensor_tensor(
                    out=d2sel[:], in0=uu[:], scalar=inv_s2_bc, in1=selr[:],
                    op0=ALU.mult, op1=ALU.add,
                )
                # contacts = exp(-100*max(mind2,1e-12)) (clamp via relu shift)
                rmin = sb2.tile([128, NT], f32, tag="rmin")
                nc.scalar.activation(rmin[:], mind2[:], AF.Relu, bias=b_negeps)
                cont = sb2.tile([128, NT], f32, tag="cont")
                nc.scalar.activation(cont[:], rmin[:], AF.Exp,
                                     bias=b_negexp, scale=-100.0)
                cdiff = sb2.tile([128, NT], f32, tag="cdiff")
                nc.vector.tensor_tensor(cdiff[:], cont[:], hc[:], op=ALU.subtract)
                jnk1 = sb2.tile([128, NT], f32, tag="jnk1")
                nc.scalar.activation(jnk1[:], cdiff[:], AF.Square,
                                     accum_out=part[:, 2 + b : 3 + b])
                # d_sel = sqrt(max(d2sel,1e-12)); err = d_sel - anc_d
                rsel = sb2.tile([128, NT], f32, tag="rsel")
                nc.scalar.activation(rsel[:], d2sel[:], AF.Relu, bias=b_negeps)
                dsel = sb2.tile([128, NT], f32, tag="dsel")
                nc.scalar.activation(dsel[:], rsel[:], AF.Sqrt, bias=b_poseps)
                ddiff = sb2.tile([128, NT], f32, tag="ddiff")
                nc.vector.tensor_tensor(
                    ddiff[:], dsel[:],
                    ch[:].rearrange("p (t s) -> p t s", s=6)[:, :, 4:5].squeeze(2),
                    op=ALU.subtract,
                )
                jnk2 = sb2.tile([128, NT], f32, tag="jnk2")
                nc.scalar.activation(jnk2[:], ddiff[:], AF.Square,
                                     accum_out=part[:, 0 + b : 1 + b])

            # ---------------- partition reduction ----------------
            psum_fin = pst.tile([8, 128], f32, tag="tp")
            nc.tensor.matmul(psum_fin[0:4, 0:1], part[:], ones,
                             start=True, stop=True)
            res4 = sb.tile([4, 1], f32, tag="res4")
            nc.scalar.activation(res4[:], psum_fin[0:4, 0:1], AF.Copy)
            nc.sync.dma_start(out_d[:], res4[:])

    nc.compile()
    return nc


def _get_program():
    if "nc" not in _CACHE:
        _CACHE["nc"] = _build_program()
    return _CACHE["nc"]


def kernel(verts, anchors, choir, hand_contacts, bps_mean, bps_scalar,
           bps_basis, _trace=False):
    from concourse.bass_utils import run_bass_kernel_spmd

    verts = np.ascontiguousarray(np.asarray(verts, np.float32))
    anchors = np.ascontiguousarray(np.asarray(anchors, np.float32))
    choir = np.ascontiguousarray(np.asarray(choir, np.float32))
    hand_contacts = np.ascontiguousarray(np.asarray(hand_contacts, np.float32))
    bps_mean = np.ascontiguousarray(np.asarray(bps_mean, np.float32))
    s = np.float32(np.asarray(bps_scalar).reshape(()))
    basis_nat = np.ascontiguousarray(
        np.asarray(bps_basis, np.float32).reshape(128, 3 * NT))

    nc = _get_program()
    VT_, NVC_ = V - 768, 6
    chc = np.concatenate(
        [choir.reshape(B, 128, 6 * NT), hand_contacts.reshape(B, 128, NT)],
        axis=2)
    obj = np.zeros((B, 128, 24), np.float32)
    obj[:, :, 0:18] = verts[:, 0:768, :].reshape(B, NVC_, 128, 3).transpose(
        0, 2, 1, 3).reshape(B, 128, 18)
    obj[:, 0:VT_, 18:21] = verts[:, 768:V, :]
    obj[:, 0:A, 21:24] = anchors
    in_maps = []
    for c in range(NCORES):
        lo, hi = BPC * c, BPC * (c + 1)
        row = np.zeros(12, np.float32)
        row[0:3] = bps_mean[lo].reshape(3)
        row[3:6] = bps_mean[lo + 1].reshape(3) if BPC > 1 else 0.0
        row[6] = np.float32(-2.0) / s
        row[7] = np.float32(1.0) / (s * s)
        row[8] = -1.0e-12
        row[9] = 1.0e-12
        row[10] = -1.0e-10
        in_maps.append({
            "chc": chc[lo:hi],
            "basis": basis_nat,
            "obj": obj[lo:hi],
            "sbc": np.tile(row, (128, 1)),
        })
    res = run_bass_kernel_spmd(nc, in_maps, list(range(NCORES)), trace=_trace)
    parts = np.stack([np.asarray(r["partials"], np.float64).reshape(4)
                      for r in res.results])
    choir_loss = parts[:, 0:BPC].sum() / (B * P)
    contact_loss = parts[:, 2 : 2 + BPC].sum() / (B * P)
    out = (np.float32(choir_loss), np.float32(contact_loss))
    if _trace:
        return out, res
    return out



# revision 22
# speedup vs baseline: 1.2165x; 1.2165x over previous
"""DualHOILoss Trainium2 kernel (8 NeuronCores, pure data parallel over batch).

Math (per batch b, point p, object o in verts):
    t_p = (basis_p + delta_p) / s + m           (u = basis + delta, w_o = o - m)
    d2[p,o] = |t_p - o|^2 = u.(-2w/s) + |w|^2 + |u|^2/s^2
computed as ONE K=5 f32r matmul per 128-point tile: lhsT rows
[ux,uy,uz,1,|u|^2/s^2], rhs rows [-2wx/s,-2wy/s,-2wz/s,|w|^2,1] so PSUM
holds d2 directly (no per-point correction pass).

Vert min (778 verts) per tile: verts split 389 (PSUM bankA) + 389 (PSUM
bankB).  ACT drains bankB pairs (2 tiles per op) to SBUF; DVE
tensor_tensor_reduce folds bankA(PSUM) against the drained copy with
op0=min, op1=min and writes the per-tile min straight into the mind slab
via accum_out (no scan-tail extraction).

The selected-anchor distance never goes through the matmul: the host
gathers the selected anchor coords per point (pure indexing); the device
computes d2_sel = |u|^2/s^2 + |w_sel|^2 - (2/s) u.w_sel elementwise on
Pool/DVE, then sqrt on ACT.  Activation tables load exactly twice (sqrt
early, exp late).  Loss partial sums accumulate via DVE
tensor_tensor_reduce accum into a [128,4] slab DMA'd out; the host does
the final partition sum.

Point tiling uses the SBUF-natural index map p = 32*q + tau (partition q,
tile tau) so every DMA is contiguous.
"""

import numpy as np

B, P, A, V = 16, 4096, 32, 778
NCORES = 8
BPC = B // NCORES      # batches per core
NT = P // 128          # 32 point tiles per batch
L = 389                # vert cols per PSUM bank (2*L == V)
NVC = 6                # full 128-vert chunks (768 verts)
VT = V - 128 * NVC     # tail verts (10)
INF = 3.0e38

_CACHE = {}


def _build_program():
    import concourse.bacc as bacc
    import concourse.mybir as mybir
    from concourse import tile

    f32 = mybir.dt.float32
    bf16 = mybir.dt.bfloat16
    AF = mybir.ActivationFunctionType
    ALU = mybir.AluOpType
    AX = mybir.AxisListType

    nc = bacc.Bacc(None, target_bir_lowering=False)

    chc_d = nc.dram_tensor("chc", [BPC, 128, 5 * NT], f32, kind="ExternalInput")
    ut_d = nc.dram_tensor("ut5", [BPC, 5, 128 * NT], bf16, kind="ExternalInput")
    basis_d = nc.dram_tensor("basis", [128, 3 * NT], f32, kind="ExternalInput")
    asel_d = nc.dram_tensor("asel", [BPC, 128, 3 * NT], f32, kind="ExternalInput")
    obj_d = nc.dram_tensor("obj", [BPC, 128, 21], f32, kind="ExternalInput")
    sbc_d = nc.dram_tensor("sbc", [128, 12], f32, kind="ExternalInput")
    out_d = nc.dram_tensor("partials", [128, 4], f32, kind="ExternalOutput")

    ident_np = np.eye(128, dtype=np.float32)
    ident_d = nc.inline_tensor(ident_np, "identm")

    with tile.TileContext(nc) as tc:
        with (
            tc.tile_pool(name="sb", bufs=1) as sb,          # persistent
            tc.tile_pool(name="psA", bufs=2, space="PSUM") as psA,
            tc.tile_pool(name="psB", bufs=2, space="PSUM") as psB,
            tc.tile_pool(name="pst", bufs=2, space="PSUM") as pst,
        ):
            sbc = sb.tile([128, 12], f32, tag="sbc")
            nc.sync.dma_start(sbc[:], sbc_d[:])
            ident = sb.tile([128, 128], f32, tag="ident")
            nc.sync.dma_start(ident[:], ident_d[:])
            basis = sb.tile([128, 3 * NT], f32, tag="basis")
            nc.sync.dma_start(basis[:], basis_d[:])

            chcs, asels, objs = [], [], []
            for b in range(BPC):
                ch = sb.tile([128, 5 * NT], f32, tag=f"chc_{b}")
                nc.sync.dma_start(ch[:], chc_d[b])
                chcs.append(ch)
                asl = sb.tile([128, 3 * NT], f32, tag=f"asel_{b}")
                nc.sync.dma_start(asl[:], asel_d[b])
                asels.append(asl)
                ob = sb.tile([128, 21], f32, tag=f"obj_{b}")
                nc.sync.dma_start(ob[:], obj_d[b])
                objs.append(ob)

            neg2s = sbc[:, 6:7]     # -2/s
            inv_s2 = sbc[:, 7:8]    # 1/s^2

            part = sb.tile([128, 4], f32, tag="part")

            # persistent per-batch buffers
            u5s, lts, rhss, minds = [], [], [], []
            for b in range(BPC):
                u5 = sb.tile([128, 5 * NT], f32, tag=f"u5_{b}")
                u5s.append(u5)
                lt = sb.tile([5, 128 * NT], bf16, tag=f"lt_{b}")
                nc.sync.dma_start(lt[:], ut_d[b])
                lts.append(lt)
                rhs = sb.tile([5, V], bf16, tag=f"rhs_{b}")
                rhss.append(rhs)
                mind = sb.tile([128, NT], f32, tag=f"mind_{b}")
                minds.append(mind)

            # drain ping-pong + scan slabs (4 tails per slab, extracted at once)
            c2bufs, junks = [], []
            for i in range(2):
                c2b = sb.tile([128, 2 * L], f32, tag=f"c2_{i}", name=f"c2_{i}")
                c2bufs.append(c2b)
                jnkb = sb.tile([128, 4 * L], f32, tag=f"junk_{i}", name=f"junk_{i}")
                junks.append(jnkb)

            # ---------------- per-batch prologue ----------------
            for b in range(BPC):
                ch = chcs[b]
                chv = ch[:].rearrange("p (t s) -> p t s", s=5)
                u5 = u5s[b]
                u5v = u5[:].rearrange("p (t s) -> p t s", s=5)
                mrep = sbc[:, 3 * b : 3 * b + 3]

                # u = basis + delta; u5 rows [ux,uy,uz,1,uu/s^2]
                nc.gpsimd.tensor_tensor(
                    u5v[:, :, 0:3],
                    chv[:, :, 0:3],
                    basis[:].rearrange("p (t d) -> p t d", d=3),
                    op=ALU.add,
                )
                usq = sb.tile([128, 3 * NT], f32, tag="usq", bufs=2)
                nc.gpsimd.tensor_tensor(
                    usq[:].rearrange("p (t d) -> p t d", d=3),
                    u5v[:, :, 0:3], u5v[:, :, 0:3], op=ALU.mult)
                uu = sb.tile([128, NT], f32, tag="uu", bufs=2)
                nc.vector.tensor_reduce(
                    uu[:], usq[:].rearrange("p (t d) -> p t d", d=3),
                    axis=AX.X, op=ALU.add)
                nc.vector.tensor_scalar_mul(u5v[:, :, 4:5].squeeze(2), uu[:], inv_s2)

                # rhs build: w = v - m, rows [-2w/s (3), |w|^2]
                obj = objs[b]
                wnat = sb.tile([128, 3 * NVC], f32, tag="wnat", bufs=2)
                nc.gpsimd.tensor_tensor(
                    wnat[:].rearrange("p (c d) -> p c d", d=3),
                    obj[:, 0:18].rearrange("p (c d) -> p c d", d=3),
                    mrep.unsqueeze(1).broadcast_to([128, NVC, 3]),
                    op=ALU.subtract,
                )
                wtail = sb.tile([VT, 3], f32, tag="wtail", bufs=2)
                nc.gpsimd.tensor_tensor(
                    wtail[:], obj[0:VT, 18:21], mrep[0:VT], op=ALU.subtract)
                s7 = sb.tile([128, 5 * NVC], f32, tag="s7", bufs=2)
                s7v = s7[:].rearrange("p (c d) -> p c d", d=5)
                st = sb.tile([VT, 5], f32, tag="st", bufs=2)
                nc.vector.tensor_scalar_mul(
                    s7v[:, :, 0:3],
                    wnat[:].rearrange("p (c d) -> p c d", d=3), neg2s)
                nc.vector.tensor_scalar_mul(st[:, 0:3], wtail[:], neg2s[0:VT])
                nc.gpsimd.memset(s7v[:, :, 4:5], 1.0)
                nc.gpsimd.memset(st[:, 4:5], 1.0)
                wsq = sb.tile([128, 3 * NVC], f32, tag="wsq", bufs=2)
                nc.gpsimd.tensor_tensor(wsq[:], wnat[:], wnat[:], op=ALU.mult)
                nc.vector.tensor_reduce(
                    s7v[:, :, 3:4].squeeze(2),
                    wsq[:].rearrange("p (c d) -> p c d", d=3),
                    axis=AX.X, op=ALU.add)
                wsqt = sb.tile([VT, 3], f32, tag="wsqt", bufs=2)
                nc.gpsimd.tensor_tensor(wsqt[:], wtail[:], wtail[:], op=ALU.mult)
                nc.vector.tensor_reduce(st[:, 3:4], wsqt[:], axis=AX.X, op=ALU.add)

                rhs = rhss[b]
                T1 = pst.tile([128, 512], f32, tag="tp")
                for c in range(4):
                    nc.tensor.transpose(
                        T1[0:5, 128 * c : 128 * (c + 1)],
                        s7[:, 5 * c : 5 * c + 5], ident[:])
                nc.scalar.activation(rhs[0:5, 0:512], T1[0:5, 0:512], AF.Copy)
                T2 = pst.tile([128, 512], f32, tag="tp")
                for c in range(4, 6):
                    nc.tensor.transpose(
                        T2[0:5, 128 * (c - 4) : 128 * (c - 3)],
                        s7[:, 5 * c : 5 * c + 5], ident[:])
                nc.tensor.transpose(T2[0:5, 256 : 256 + VT], st[:], ident[0:VT, 0:VT])
                nc.scalar.activation(rhs[0:5, 512:V], T2[0:5, 0 : 256 + VT], AF.Copy)

                # choir branch: d2_sel from host-gathered anchor coords
                asl = asels[b]
                wsel = sb.tile([128, 3 * NT], f32, tag="wsel", bufs=2)
                nc.gpsimd.tensor_tensor(
                    wsel[:].rearrange("p (t d) -> p t d", d=3),
                    asl[:].rearrange("p (t d) -> p t d", d=3),
                    mrep.unsqueeze(1).broadcast_to([128, NT, 3]),
                    op=ALU.subtract,
                )
                usel = sb.tile([128, 3 * NT], f32, tag="usel", bufs=2)
                nc.gpsimd.tensor_tensor(
                    usel[:].rearrange("p (t d) -> p t d", d=3),
                    u5v[:, :, 0:3],
                    wsel[:].rearrange("p (t d) -> p t d", d=3),
                    op=ALU.mult,
                )
                uw = sb.tile([128, NT], f32, tag="uw", bufs=2)
                nc.vector.tensor_reduce(
                    uw[:], usel[:].rearrange("p (t d) -> p t d", d=3),
                    axis=AX.X, op=ALU.add)
                wsq2 = sb.tile([128, 3 * NT], f32, tag="wsq2", bufs=2)
                nc.gpsimd.tensor_tensor(wsq2[:], wsel[:], wsel[:], op=ALU.mult)
                w2 = sb.tile([128, NT], f32, tag="w2", bufs=2)
                nc.vector.tensor_reduce(
                    w2[:], wsq2[:].rearrange("p (t d) -> p t d", d=3),
                    axis=AX.X, op=ALU.add)
                d2s = sb.tile([128, NT], f32, tag="d2s", bufs=2)
                nc.vector.scalar_tensor_tensor(
                    out=d2s[:], in0=uw[:], scalar=neg2s, in1=w2[:],
                    op0=ALU.mult, op1=ALU.add)
                nc.gpsimd.tensor_tensor(
                    d2s[:], d2s[:], u5v[:, :, 4:5].squeeze(2), op=ALU.add)
                rsel = sb.tile([128, NT], f32, tag="rsel", bufs=2)
                nc.vector.tensor_scalar_max(rsel[:], d2s[:], 1.0e-12)
                dsel = sb.tile([128, NT], f32, tag="dsel", bufs=2)
                nc.scalar.activation(dsel[:], rsel[:], AF.Sqrt)
                ddiff = sb.tile([128, NT], f32, tag="ddiff", bufs=2)
                nc.gpsimd.tensor_tensor(
                    ddiff[:], dsel[:], chv[:, :, 3:4].squeeze(2), op=ALU.subtract)
                jnk = sb.tile([128, NT], f32, tag="jnkd", bufs=2)
                nc.scalar.activation(jnk[:], ddiff[:], AF.Square,
                                     accum_out=part[:, b : b + 1])

            # ---------------- tile loops ----------------
            for b in range(BPC):
                lt = lts[b]
                rhs = rhss[b]
                mind = minds[b]
                for kp in range(NT // 2):
                    c2 = c2bufs[kp % 2]
                    c2v = c2[:].rearrange("p (j w) -> p j w", j=2)
                    jb = junks[(kp // 2) % 2]
                    ptB = psB.tile([128, 1024], f32, tag="ptB")
                    ptAs = []
                    for j in range(2):
                        t = 2 * kp + j
                        ltT = lt[:, 128 * t : 128 * (t + 1)]
                        ptA = psA.tile([128, 512], f32, tag="ptA")
                        ptAs.append(ptA)
                        nc.tensor.matmul(ptA[:, 0:L], ltT, rhs[:, 0:L],
                                         start=True, stop=True)
                        nc.tensor.matmul(ptB[:, 512 * j : 512 * j + L], ltT,
                                         rhs[:, L:V], start=True, stop=True)
                    nc.scalar.activation(
                        c2v[:, :, :],
                        ptB[:].rearrange("p (j w) -> p j w", j=2)[:, :, 0:L],
                        AF.Copy,
                    )
                    for j in range(2):
                        s = 2 * (kp % 2) + j
                        nc.vector.tensor_tensor_scan(
                            out=jb[:, L * s : L * (s + 1)],
                            data0=ptAs[j][:, 0:L], data1=c2v[:, j, :],
                            initial=INF, op0=ALU.min, op1=ALU.min)
                    if kp % 2 == 1:
                        nc.gpsimd.tensor_copy(
                            mind[:, 2 * kp - 2 : 2 * kp + 2],
                            jb[:].rearrange("p (s w) -> p s w", w=L)[:, :, L - 1],
                        )

                # contact tail for this batch
                cont = sb.tile([128, NT], f32, tag="cont", bufs=2)
                nc.scalar.activation(cont[:], mind[:], AF.Exp, scale=-100.0)
                cdiff = sb.tile([128, NT], f32, tag="cdiff", bufs=2)
                nc.gpsimd.tensor_tensor(
                    cdiff[:], cont[:],
                    chcs[b][:].rearrange("p (t s) -> p t s", s=5)[:, :, 4:5]
                        .squeeze(2),
                    op=ALU.subtract)
                jnk2 = sb.tile([128, NT], f32, tag="jnkc", bufs=2)
                nc.scalar.activation(jnk2[:], cdiff[:], AF.Square,
                                     accum_out=part[:, 2 + b : 3 + b])

            nc.sync.dma_start(out_d[:], part[:])

    nc.compile()
    return nc


def _get_program():
    if "nc" not in _CACHE:
        _CACHE["nc"] = _build_program()
    return _CACHE["nc"]


def kernel(verts, anchors, choir, hand_contacts, bps_mean, bps_scalar,
           bps_basis, _trace=False):
    from concourse.bass_utils import run_bass_kernel_spmd

    verts = np.ascontiguousarray(np.asarray(verts, np.float32))
    anchors = np.ascontiguousarray(np.asarray(anchors, np.float32))
    choir = np.ascontiguousarray(np.asarray(choir, np.float32))
    hand_contacts = np.ascontiguousarray(np.asarray(hand_contacts, np.float32))
    bps_mean = np.ascontiguousarray(np.asarray(bps_mean, np.float32))
    s = np.float32(np.asarray(bps_scalar).reshape(()))
    basis_nat = np.ascontiguousarray(
        np.asarray(bps_basis, np.float32).reshape(128, 3 * NT))

    nc = _get_program()
    # per-point slab [dx,dy,dz,anc_d,hc] with p = 32*q + tau map
    chc = np.concatenate(
        [choir[:, :, 1:4], choir[:, :, 4:5], hand_contacts[:, :, None]],
        axis=2).reshape(B, 128, 5 * NT)
    idx = choir[:, :, 5].astype(np.int64)
    asel = np.take_along_axis(anchors, idx[:, :, None], axis=1)  # (B,P,3)
    asel = asel.reshape(B, 128, 3 * NT)
    # lhsT layout: ut5[b, r, 128*t + q] = row r of point p = 32*q + t
    u = basis_nat.reshape(128, NT, 3)[None] + choir[:, :, 1:4].reshape(
        B, 128, NT, 3)                                           # (B,128,NT,3)
    uu2 = (u * u).sum(-1) / (s * s)                              # (B,128,NT)
    import ml_dtypes
    ut5 = np.empty((B, 5, NT, 128), np.float32)
    ut5[:, 0:3] = u.transpose(0, 3, 2, 1)
    ut5[:, 3] = 1.0
    ut5[:, 4] = uu2.transpose(0, 2, 1)
    ut5 = ut5.reshape(B, 5, 128 * NT).astype(ml_dtypes.bfloat16)
    obj = np.zeros((B, 128, 21), np.float32)
    obj[:, :, 0:18] = verts[:, 0:768, :].reshape(B, NVC, 128, 3).transpose(
        0, 2, 1, 3).reshape(B, 128, 18)
    obj[:, 0:VT, 18:21] = verts[:, 768:V, :]
    in_maps = []
    for c in range(NCORES):
        lo = BPC * c
        row = np.zeros(12, np.float32)
        row[0:3] = bps_mean[lo].reshape(3)
        row[3:6] = bps_mean[lo + 1].reshape(3) if BPC > 1 else 0.0
        row[6] = np.float32(-2.0) / s
        row[7] = np.float32(1.0) / (s * s)
        in_maps.append({
            "chc": chc[lo : lo + BPC],
            "ut5": ut5[lo : lo + BPC],
            "basis": basis_nat,
            "asel": asel[lo : lo + BPC],
            "obj": obj[lo : lo + BPC],
            "sbc": np.tile(row, (128, 1)),
        })
    res = run_bass_kernel_spmd(nc, in_maps, list(range(NCORES)))
    parts = np.stack([np.asarray(r["partials"], np.float64).reshape(128, 4)
                      for r in res.results])
    psum = parts.sum(axis=(0, 1))
    choir_loss = (psum[0] + psum[1]) / (B * P)
    contact_loss = (psum[2] + psum[3]) / (B * P)
    out = (np.float32(choir_loss), np.float32(contact_loss))
    if _trace:
        return out, res
    return out


# revision 24
# speedup vs baseline: 1.2715x; 1.0452x over previous
"""DualHOILoss Trainium2 kernel (8 NeuronCores, pure data parallel over batch).

Math (per batch b, point p, vert o):
    t_p = (basis_p + delta_p) / s + m           (u = basis + delta, w_o = o - m)
    d2[p,o] = |t_p - o|^2 = u.(-2w/s) + |w|^2 + |u|^2/s^2
computed as ONE K=5 bf16 matmul per 128-point tile: lhsT rows
[ux,uy,uz,1,|u|^2/s^2], rhs rows [-2wx/s,-2wy/s,-2wz/s,|w|^2,1] so PSUM
holds d2 directly.  The host packs the (tiny) coefficient tensors: lhsT in
transposed matmul layout, rhs rows, and the per-point u/|u|^2 slab in
partition layout; the device does all the O(P*V) work.

Vert min (778 verts) per tile: verts split 389 (PSUM bankA) + 389 (PSUM
bankB).  ACT drains bankB pairs (2 tiles per ACT op) to SBUF; one DVE
tensor_tensor_scan per tile folds bankA (PSUM) against the drained copy
(min,min) - 2 streams per DVE cycle, the best min rate on the core.  Scan
tails land in 4 rotating slabs; Pool extracts 4 tails per strided copy.

The selected-anchor distance never goes through the matmul: the host
gathers the selected anchor coords per point (pure indexing); the device
computes d2_sel = |u|^2/s^2 + |w_sel|^2 - (2/s) u.w_sel elementwise on
Pool/DVE, then sqrt on ACT.  Activation tables load exactly twice (sqrt
during the DMA window, exp at batch-0 tail).  Loss partials accumulate via
ACT Square+accum into a [128,4] slab; the host does the final partition
sum.

Point tiling uses the SBUF-natural index map p = 32*q + tau (partition q,
tile tau) so every DMA is contiguous.
"""

import numpy as np

B, P, A, V = 16, 4096, 32, 778
NCORES = 8
BPC = B // NCORES      # batches per core
NT = P // 128          # 32 point tiles per batch
L = 389                # vert cols per PSUM bank (2*L == V)
INF = 3.0e38

_CACHE = {}


def _build_program():
    import concourse.bacc as bacc
    import concourse.mybir as mybir
    from concourse import tile

    f32 = mybir.dt.float32
    bf16 = mybir.dt.bfloat16
    AF = mybir.ActivationFunctionType
    ALU = mybir.AluOpType
    AX = mybir.AxisListType

    nc = bacc.Bacc(None, target_bir_lowering=False)

    ut_d = nc.dram_tensor("ut5", [BPC, 5, 128 * NT], bf16, kind="ExternalInput")
    rhs_d = nc.dram_tensor("rhs5", [BPC, 5, V], bf16, kind="ExternalInput")
    uch_d = nc.dram_tensor("uch", [BPC, 128, 4 * NT], f32, kind="ExternalInput")
    chc_d = nc.dram_tensor("chc", [BPC, 128, 2 * NT], f32, kind="ExternalInput")
    asel_d = nc.dram_tensor("asel", [BPC, 128, 3 * NT], f32, kind="ExternalInput")
    sbc_d = nc.dram_tensor("sbc", [128, 12], f32, kind="ExternalInput")
    out_d = nc.dram_tensor("partials", [128, 4], f32, kind="ExternalOutput")

    with tile.TileContext(nc) as tc:
        with (
            tc.tile_pool(name="sb", bufs=1) as sb,          # persistent
            tc.tile_pool(name="psA", bufs=3, space="PSUM") as psA,
            tc.tile_pool(name="psB", bufs=2, space="PSUM") as psB,
        ):
            # ---- DMAs spread over 3 HWDGE queues, tile-loop-critical first
            sbc = sb.tile([128, 12], f32, tag="sbc")
            nc.gpsimd.dma_start(sbc[:], sbc_d[:])

            lts, rhss = [], []
            for b in range(BPC):
                lt = sb.tile([5, 128 * NT], bf16, tag=f"lt_{b}", name=f"lt_{b}")
                h = 64 * NT
                nc.sync.dma_start(lt[:, 0:h], ut_d[b][:, 0:h])
                nc.scalar.dma_start(lt[:, h:], ut_d[b][:, h:])
                lts.append(lt)
                rhs = sb.tile([5, V], bf16, tag=f"rhs_{b}", name=f"rhs_{b}")
                nc.gpsimd.dma_start(rhs[:], rhs_d[b])
                rhss.append(rhs)

            uchs, chcs, asels = [], [], []
            for b in range(BPC):
                uc = sb.tile([128, 4 * NT], f32, tag=f"uch_{b}", name=f"uch_{b}")
                nc.gpsimd.dma_start(uc[:], uch_d[b])
                uchs.append(uc)
                asl = sb.tile([128, 3 * NT], f32, tag=f"asel_{b}", name=f"as_{b}")
                nc.gpsimd.dma_start(asl[:], asel_d[b])
                asels.append(asl)
                ch = sb.tile([128, 2 * NT], f32, tag=f"chc_{b}", name=f"chc_{b}")
                nc.scalar.dma_start(ch[:], chc_d[b])
                chcs.append(ch)

            neg2s = sbc[:, 6:7]     # -2/s

            part = sb.tile([128, 4], f32, tag="part")
            minds = []
            for b in range(BPC):
                mind = sb.tile([128, NT], f32, tag=f"mind_{b}", name=f"mind_{b}")
                minds.append(mind)

            # drain ping-pong + 4 rotating scan slabs (4 tails per slab)
            c2bufs, junks = [], []
            for i in range(2):
                c2b = sb.tile([128, 2 * L], f32, tag=f"c2_{i}", name=f"c2_{i}")
                c2bufs.append(c2b)
            for i in range(4):
                jnkb = sb.tile([128, 4 * L], f32, tag=f"junk_{i}", name=f"jk_{i}")
                junks.append(jnkb)

            # ---------------- choir branch (no matmul; sqrt table early) ----
            for b in range(BPC):
                ucv = uchs[b][:].rearrange("p (t s) -> p t s", s=4)
                chv = chcs[b][:].rearrange("p (t s) -> p t s", s=2)
                mrep = sbc[:, 3 * b : 3 * b + 3]

                wsel = sb.tile([128, 3 * NT], f32, tag="wsel", bufs=2)
                nc.gpsimd.tensor_tensor(
                    wsel[:].rearrange("p (t d) -> p t d", d=3),
                    asels[b][:].rearrange("p (t d) -> p t d", d=3),
                    mrep.unsqueeze(1).broadcast_to([128, NT, 3]),
                    op=ALU.subtract,
                )
                usel = sb.tile([128, 3 * NT], f32, tag="usel", bufs=2)
                nc.gpsimd.tensor_tensor(
                    usel[:].rearrange("p (t d) -> p t d", d=3),
                    ucv[:, :, 0:3],
                    wsel[:].rearrange("p (t d) -> p t d", d=3),
                    op=ALU.mult,
                )
                uw = sb.tile([128, NT], f32, tag="uw", bufs=2)
                nc.vector.tensor_reduce(
                    uw[:], usel[:].rearrange("p (t d) -> p t d", d=3),
                    axis=AX.X, op=ALU.add)
                wsq2 = sb.tile([128, 3 * NT], f32, tag="wsq2", bufs=2)
                nc.gpsimd.tensor_tensor(wsq2[:], wsel[:], wsel[:], op=ALU.mult)
                w2 = sb.tile([128, NT], f32, tag="w2", bufs=2)
                nc.vector.tensor_reduce(
                    w2[:], wsq2[:].rearrange("p (t d) -> p t d", d=3),
                    axis=AX.X, op=ALU.add)
                d2s = sb.tile([128, NT], f32, tag="d2s", bufs=2)
                nc.vector.scalar_tensor_tensor(
                    out=d2s[:], in0=uw[:], scalar=neg2s, in1=w2[:],
                    op0=ALU.mult, op1=ALU.add)
                nc.gpsimd.tensor_tensor(
                    d2s[:], d2s[:], ucv[:, :, 3:4].squeeze(2), op=ALU.add)
                rsel = sb.tile([128, NT], f32, tag="rsel", bufs=2)
                nc.vector.tensor_scalar_max(rsel[:], d2s[:], 1.0e-12)
                dsel = sb.tile([128, NT], f32, tag="dsel", bufs=2)
                nc.scalar.activation(dsel[:], rsel[:], AF.Sqrt)
                ddiff = sb.tile([128, NT], f32, tag="ddiff", bufs=2)
                nc.gpsimd.tensor_tensor(
                    ddiff[:], dsel[:], chv[:, :, 0:1].squeeze(2), op=ALU.subtract)
                jnk = sb.tile([128, NT], f32, tag="jnkd", bufs=2)
                nc.scalar.activation(jnk[:], ddiff[:], AF.Square,
                                     accum_out=part[:, b : b + 1])

            # ---------------- tile loops ----------------
            for b in range(BPC):
                lt = lts[b]
                rhs = rhss[b]
                mind = minds[b]
                for kp in range(NT // 2):
                    c2 = c2bufs[kp % 2]
                    c2v = c2[:].rearrange("p (j w) -> p j w", j=2)
                    jb = junks[(kp // 2) % 4]
                    ptB = psB.tile([128, 1024], f32, tag="ptB")
                    ptAs = []
                    for j in range(2):
                        t = 2 * kp + j
                        ltT = lt[:, 128 * t : 128 * (t + 1)]
                        ptA = psA.tile([128, 512], f32, tag="ptA")
                        ptAs.append(ptA)
                        nc.tensor.matmul(ptA[:, 0:L], ltT, rhs[:, 0:L],
                                         start=True, stop=True)
                        nc.tensor.matmul(ptB[:, 512 * j : 512 * j + L], ltT,
                                         rhs[:, L:V], start=True, stop=True)
                    nc.scalar.activation(
                        c2v[:, :, :],
                        ptB[:].rearrange("p (j w) -> p j w", j=2)[:, :, 0:L],
                        AF.Copy,
                    )
                    for j in range(2):
                        s = 2 * (kp % 2) + j
                        nc.vector.tensor_tensor_scan(
                            out=jb[:, L * s : L * (s + 1)],
                            data0=ptAs[j][:, 0:L], data1=c2v[:, j, :],
                            initial=INF, op0=ALU.min, op1=ALU.min)
                    if kp % 2 == 1:
                        nc.gpsimd.tensor_copy(
                            mind[:, 2 * kp - 2 : 2 * kp + 2],
                            jb[:].rearrange("p (s w) -> p s w", w=L)[:, :, L - 1],
                        )

                # contact tail for this batch
                cont = sb.tile([128, NT], f32, tag="cont", bufs=2)
                nc.scalar.activation(cont[:], mind[:], AF.Exp, scale=-100.0)
                cdiff = sb.tile([128, NT], f32, tag="cdiff", bufs=2)
                nc.gpsimd.tensor_tensor(
                    cdiff[:], cont[:],
                    chcs[b][:].rearrange("p (t s) -> p t s", s=2)[:, :, 1:2]
                        .squeeze(2),
                    op=ALU.subtract)
                jnk2 = sb.tile([128, NT], f32, tag="jnkc", bufs=2)
                nc.scalar.activation(jnk2[:], cdiff[:], AF.Square,
                                     accum_out=part[:, 2 + b : 3 + b])

            nc.sync.dma_start(out_d[:], part[:])

    nc.compile()
    return nc


def _get_program():
    if "nc" not in _CACHE:
        _CACHE["nc"] = _build_program()
    return _CACHE["nc"]


def _pack(verts, anchors, choir, hand_contacts, bps_mean, bps_scalar,
          bps_basis):
    """Host-side layout packing of the small coefficient tensors."""
    import ml_dtypes
    verts = np.ascontiguousarray(np.asarray(verts, np.float32))
    anchors = np.ascontiguousarray(np.asarray(anchors, np.float32))
    choir = np.ascontiguousarray(np.asarray(choir, np.float32))
    hand_contacts = np.ascontiguousarray(np.asarray(hand_contacts, np.float32))
    bps_mean = np.asarray(bps_mean, np.float32).reshape(B, 3)
    s = np.float32(np.asarray(bps_scalar).reshape(()))
    basis = np.asarray(bps_basis, np.float32).reshape(P, 3)

    # per-point target slab [anc_d, hc] with p = 32q + tau map
    chc = np.concatenate(
        [choir[:, :, 4:5], hand_contacts[:, :, None]], axis=2,
    ).reshape(B, 128, 2 * NT)
    idx = choir[:, :, 5].astype(np.int64)
    asel = np.take_along_axis(anchors, idx[:, :, None], axis=1)
    asel = asel.reshape(B, 128, 3 * NT)

    u = basis[None] + choir[:, :, 1:4]                       # (B,P,3)
    uu2 = (u * u).sum(-1) / (s * s)                          # (B,P)
    # lhsT layout: ut5[b, r, 128*t + q] = row r of point p = 32*q + t
    ur = u.reshape(B, 128, NT, 3)
    ut5 = np.empty((B, 5, NT, 128), np.float32)
    ut5[:, 0:3] = ur.transpose(0, 3, 2, 1)
    ut5[:, 3] = 1.0
    ut5[:, 4] = uu2.reshape(B, 128, NT).transpose(0, 2, 1)
    ut5 = ut5.reshape(B, 5, 128 * NT).astype(ml_dtypes.bfloat16)
    # partition-layout u slab [ux,uy,uz,uu/s^2] for the choir branch
    uch = np.concatenate(
        [ur, uu2.reshape(B, 128, NT)[:, :, :, None]], axis=3,
    ).reshape(B, 128, 4 * NT)
    # rhs rows [-2w/s (3), |w|^2, 1]
    w = verts - bps_mean[:, None, :]                         # (B,V,3)
    rhs5 = np.empty((B, 5, V), np.float32)
    rhs5[:, 0:3] = (w * (np.float32(-2.0) / s)).transpose(0, 2, 1)
    rhs5[:, 3] = (w * w).sum(-1)
    rhs5[:, 4] = 1.0
    rhs5 = rhs5.astype(ml_dtypes.bfloat16)

    in_maps = []
    for c in range(NCORES):
        lo = BPC * c
        row = np.zeros(12, np.float32)
        row[0:3] = bps_mean[lo]
        row[3:6] = bps_mean[lo + 1] if BPC > 1 else 0.0
        row[6] = np.float32(-2.0) / s
        in_maps.append({
            "ut5": ut5[lo : lo + BPC],
            "rhs5": rhs5[lo : lo + BPC],
            "uch": uch[lo : lo + BPC],
            "chc": chc[lo : lo + BPC],
            "asel": asel[lo : lo + BPC],
            "sbc": np.tile(row, (128, 1)),
        })
    return in_maps


def kernel(verts, anchors, choir, hand_contacts, bps_mean, bps_scalar,
           bps_basis, _trace=False):
    from concourse.bass_utils import run_bass_kernel_spmd

    nc = _get_program()
    in_maps = _pack(verts, anchors, choir, hand_contacts, bps_mean,
                    bps_scalar, bps_basis)
    res = run_bass_kernel_spmd(nc, in_maps, list(range(NCORES)))
    parts = np.stack([np.asarray(r["partials"], np.float64).reshape(128, 4)
                      for r in res.results])
    psum = parts.sum(axis=(0, 1))
    choir_loss = (psum[0] + psum[1]) / (B * P)
    contact_loss = (psum[2] + psum[3]) / (B * P)
    out = (np.float32(choir_loss), np.float32(contact_loss))
    if _trace:
        return out, res
    return out
